# revision 28
# baseline (speedup 1.0000x reference)
"""Trainium2 Bass kernel for nn_MidBlock (ResNet -> Attention -> ResNet).

Data-parallel over batch: 16 images -> 8 cores x 2 images.
Convs use Winograd F(2x2,3x3): conv3x3 becomes 16 per-tile-position
[C_in x C_out] matmuls over 256 tiles/image, cutting tensor-engine
columns 2.4x vs direct conv. All matmul data in fp16 (fp32 accumulate).
Winograd input transforms run on DVE via strided views; the output
transform drains PSUM through ACT copies + DVE combines into a
contiguous linear Y buffer (GroupNorm sums fused via accum_out), and
the GroupNorm SiLU on ACT scatters Y back into padded-frame layout.
Softmax runs on transposed scores (kv on partitions): row sums come
from a ones-matmul broadcast, so no PE transposes are needed.
"""

import contextlib

import numpy as np

import concourse.bacc as bacc
import concourse.bass as bass
import concourse.tile as tile
from concourse import mybir
from concourse.bass_utils import run_bass_kernel_spmd

F32 = mybir.dt.float32
F16 = mybir.dt.float16
AF = mybir.ActivationFunctionType
OP = mybir.AluOpType
AX = mybir.AxisListType

N_CORES = 8
C = 512
B = 16
HH = 32
WW = 32
B_LOC = B // N_CORES  # 2 images per core
NCHI = 4  # channel blocks of 128
FW = 34  # padded frame width
FR = FW * FW  # 1156
PFREE = NCHI * FR  # 4624
EPS = 1e-6
GCNT = 16 * HH * WW  # elements per group

# consts tile column map (CT [128, 80] fp32)
CB = {"r1c1": 0, "r1c2": 4, "r2c1": 8, "r2c2": 12}
GN_COLS = {"r1g1": (16, 20), "r1g2": (24, 28), "att": (32, 36),
           "r2g1": (40, 44), "r2g2": (48, 52)}
A_COL = 56
QB_COL, KB_COL, VB_COL, PB_COL = 64, 68, 72, 76


def _fb(chi):
    return chi * FR


def _valid(t, chi):
    """[128, 32, 32] view of valid pixels of frame chi in per-image tile t."""
    s = t[:, _fb(chi) + FW: _fb(chi) + FW + 32 * FW]
    return s.rearrange("p (r w) -> p r w", w=FW)[:, :, 1:33]


def _build(num_devices):
    nc = bacc.Bacc("TRN2", target_bir_lowering=False, debug=False,
                   num_devices=num_devices)
    x_pad = nc.dram_tensor("x_pad", [128, B_LOC, PFREE], F16,
                           kind="ExternalInput").ap()
    wc = {k: nc.dram_tensor(f"w_{k}", [128, 4, 2, 8, NCHI, 128], F16,
                            kind="ExternalInput").ap()
          for k in ("r1c1", "r1c2", "r2c1", "r2c2")}
    wqkvp = nc.dram_tensor("wqkvp", [128, 4, NCHI, C], F16,
                           kind="ExternalInput").ap()
    ct_d = nc.dram_tensor("consts", [128, 80], F32, kind="ExternalInput").ap()
    atm_d = nc.dram_tensor("atm", [8, 128], F32, kind="ExternalInput").ap()
    out_d = nc.dram_tensor("out", [128, B_LOC, NCHI, 1024], F16,
                           kind="ExternalOutput").ap()

    with tile.TileContext(nc) as tc, contextlib.ExitStack() as ctx:
        pers = ctx.enter_context(tc.tile_pool(name="pers", bufs=1))
        scr = ctx.enter_context(tc.tile_pool(name="scr", bufs=1))
        wpool = ctx.enter_context(tc.tile_pool(name="wpool", bufs=1))
        cpool = ctx.enter_context(tc.tile_pool(name="cpool", bufs=1))
        spool = ctx.enter_context(tc.tile_pool(name="spool", bufs=1))
        apool = ctx.enter_context(tc.tile_pool(name="apool", bufs=1))
        vpool = ctx.enter_context(tc.tile_pool(name="vpool", bufs=1))
        psum = ctx.enter_context(tc.tile_pool(name="psum", bufs=1, space="PSUM"))

        psctr = [0]

        def ps_slot():
            t = psum.tile([128, 512], F32, tag=f"m{psctr[0] % 6}",
                          name=f"ps{psctr[0]}")
            psctr[0] += 1
            return t

        def small_ps():
            return psum.tile([128, 512], F32, tag="tp", name="tp", bufs=2)

        # ---- persistent residual frames + input DMAs ----
        XF = [pers.tile([128, PFREE], F16, tag=f"xf{b}", name=f"xf{b}")
              for b in range(B_LOC)]
        for b, eng in ((0, nc.sync), (1, nc.gpsimd)):
            for chi in range(NCHI):
                eng.dma_start(out=XF[b][:, _fb(chi):_fb(chi) + FR],
                              in_=x_pad[:, b, _fb(chi):_fb(chi) + FR])

        CT = cpool.tile([128, 80], F32, tag="ct", name="ct")
        nc.sync.dma_start(out=CT, in_=ct_d)
        ATM = cpool.tile([8, 128], F32, tag="atm", name="atm")
        nc.sync.dma_start(out=ATM, in_=atm_d)
        WA = cpool.tile([128, 4, NCHI, C], F16, tag="wqkvp", name="wqkvp")
        nc.gpsimd.dma_start(out=WA, in_=wqkvp)
        ONES = cpool.tile([128, 128], F16, tag="ones", name="ones")
        nc.vector.memset(ONES, 1.0)
        GAR = scr.tile([128, FR], F16, tag="gar", name="gar")
        garv = GAR[:, 0:1024].rearrange("p (r w) -> p r w", w=32)

        wctr = [0]
        wdmae = [nc.sync, nc.gpsimd]

        # ---------------- Winograd input transform ----------------
        # c-planes stored [t][(c i)][bb][j]: row-parity-major so every view
        # below is a legal 2-free-dim AP. V-planes stored [a][(chi i bb j)]
        # (bb interleaved); the matmul reads strided [i, j] windows.
        def make_V(img, src, vp, pfx):
            cp = scr.tile([128, 2, 4352], F16, tag="cpl",
                          name=f"cp_{pfx}{img}")
            sv = src[:, 0:NCHI * FR].rearrange(
                "p (ci t j s) -> p ci t j s", ci=4 * 17, t=2, j=17, s=2)
            for t in range(2):
                A0 = sv[:, :, t, 0:16, 0]   # col 2j
                A1 = sv[:, :, t, 1:17, 0]   # col 2j+2
                B0 = sv[:, :, t, 0:16, 1]   # col 2j+1
                B1 = sv[:, :, t, 1:17, 1]   # col 2j+3

                def cv(n, t=t):
                    return cp[:, t].rearrange(
                        "p (ci bb j) -> p ci bb j", ci=68, bb=4,
                        j=16)[:, :, n, :]

                eng = nc.vector if t == 0 else nc.gpsimd
                eng.tensor_sub(out=cv(0), in0=A0, in1=A1)
                eng.tensor_add(out=cv(1), in0=B0, in1=A1)
                eng.tensor_sub(out=cv(2), in0=A1, in1=B0)
                eng.tensor_sub(out=cv(3), in0=B0, in1=B1)
            # H-direction: rows 2ti(+1,+2,+3) = (t, i windows); all four
            # bb planes and all chi processed in one op per output row a.
            ce = [cp[:, t].rearrange("p (c ibj) -> p c ibj", c=4, ibj=1088)
                  for t in range(2)]
            E0 = ce[0][:, :, 0:1024]
            E1 = ce[0][:, :, 64:1088]
            O0 = ce[1][:, :, 0:1024]
            O1 = ce[1][:, :, 64:1088]

            def vv(a):
                return vp[:, a].rearrange("p (c ibj) -> p c ibj", c=4,
                                          ibj=1024)

            nc.vector.tensor_sub(out=vv(0), in0=E0, in1=E1)
            nc.vector.tensor_add(out=vv(1), in0=O0, in1=E1)
            nc.vector.tensor_sub(out=vv(2), in0=E1, in1=O0)
            nc.vector.tensor_sub(out=vv(3), in0=O0, in1=O1)

        def vrhs(vp, a, bb, chi):
            """[128,16,16] strided matmul rhs for tile position (a,bb)."""
            return vp[:, a].rearrange("p (c i bb j) -> p c i bb j",
                                      c=4, i=16, bb=4, j=16)[:, chi, :, bb, :]

        def frame_view(f, co, p, q):
            """[128,16,16] strided view of output positions (p,q) of tiles."""
            f5 = f[:, _fb(co):_fb(co) + FR].rearrange(
                "pp (i t j s) -> pp i t j s", i=17, t=2, j=17, s=2)
            ri = slice(0, 16) if p == 0 else slice(1, 17)
            rj = slice(0, 16) if q == 0 else slice(1, 17)
            return f5[:, ri, 1 - p, rj, 1 - q]

        def ring_zero(f):
            """Zero the 34x34 padding ring of each chi frame (Pool engine)."""
            for chi in range(NCHI):
                base = _fb(chi)
                nc.gpsimd.memset(f[:, base:base + FW], 0.0)
                nc.gpsimd.memset(f[:, base + 33 * FW:base + 34 * FW], 0.0)
                colv = f[:, base + FW:base + 33 * FW].rearrange(
                    "p (r w) -> p r w", w=FW)
                nc.gpsimd.memset(colv[:, :, 0:1], 0.0)
                nc.gpsimd.memset(colv[:, :, 33:34], 0.0)

        # ---------------- Winograd conv (one image) ----------------
        # Output goes to a contiguous linear buffer Y [128, co, 2q+p, 256];
        # the GN SiLU pass (ACT) scatters it into frame layout afterwards.
        def conv_img(key, img, vp, outf, ss):
            bcol = CB[key]
            for co in range(4):
                tp_t = spool.tile([128, 2, 4, 256], F16, tag="tpn",
                                  name=f"t_{key}{img}{co}", bufs=3)
                for h in range(2):  # b-pair: two accum groups share a bank
                    w = wpool.tile([128, 8, NCHI, 128], F16,
                                   tag=f"w{wctr[0] % 2}",
                                   name=f"w_{key}{img}{co}{h}")
                    wdmae[wctr[0] % 2].dma_start(out=w, in_=wc[key][:, co, h])
                    wctr[0] += 1
                    slots = [ps_slot() for _ in range(4)]
                    for a in range(4):
                        for bb in range(2):
                            b = 2 * h + bb
                            for chi in range(NCHI):
                                nc.tensor.matmul(
                                    slots[a][:, bb * 256:(bb + 1) * 256],
                                    w[:, 2 * a + bb, chi, :],
                                    vrhs(vp, a, b, chi),
                                    start=chi == 0, stop=chi == 3)
                    sl = [s.rearrange("pp (a j) -> pp a j", a=2)
                          for s in slots]
                    # DVE reads at most one PSUM input per op: stage m1
                    # through an ACT copy to SBUF.
                    uu = spool.tile([128, 2, 256], F16, tag="u1",
                                    name=f"u_{key}{img}{co}{h}", bufs=2)
                    nc.scalar.copy(out=uu, in_=sl[1])
                    t0 = tp_t[:, 0, 2 * h:2 * h + 2, :]
                    t1 = tp_t[:, 1, 2 * h:2 * h + 2, :]
                    nc.vector.tensor_add(out=t0, in0=uu, in1=sl[0])
                    nc.vector.tensor_add(out=t0, in0=t0, in1=sl[2])
                    nc.vector.tensor_sub(out=t1, in0=uu, in1=sl[2])
                    nc.vector.tensor_sub(out=t1, in0=t1, in1=sl[3])
                bias = CT[:, bcol + co: bcol + co + 1]
                for q in range(2):
                    yt = spool.tile([128, 2, 256], F16, tag="yt",
                                    name=f"yt_{key}{img}{co}{q}", bufs=2)
                    if q == 0:
                        nc.vector.tensor_add(out=yt, in0=tp_t[:, 0:2, 0, :],
                                             in1=tp_t[:, 0:2, 1, :])
                    else:
                        nc.vector.tensor_sub(out=yt, in0=tp_t[:, 0:2, 1, :],
                                             in1=tp_t[:, 0:2, 2, :])
                    b3 = 2 if q == 0 else 3
                    op1 = OP.add if q == 0 else OP.subtract
                    for p in range(2):
                        k = 2 * q + p
                        ytv = yt[:, p].rearrange("pp (i j) -> pp i j", j=16)
                        t3 = tp_t[:, p, b3, :].rearrange(
                            "pp (i j) -> pp i j", j=16)
                        nc.vector.scalar_tensor_tensor(
                            out=frame_view(outf, co, p, q), in0=ytv,
                            scalar=bias, in1=t3, op0=OP.add, op1=op1,
                            accum_out=ss[:, co, k:k + 1])

        # ---------------- group norm ----------------
        def gn_finalize(gkey, ST):
            """Common GN tail: ST = [sums|sumsq] -> per-channel (s, t)."""
            gcol, bcol = GN_COLS[gkey]
            G = small_ps()
            nc.tensor.matmul(G[:8, :8], CT[:, A_COL:A_COL + 8], ST, start=True,
                             stop=True)
            SG = spool.tile([8, 8], F32, tag="sg", name="sg", bufs=4)
            T8 = spool.tile([8, 4], F32, tag="t8", name="t8", bufs=4)
            nc.vector.tensor_scalar_mul(out=SG, in0=G[:8, :8],
                                        scalar1=1.0 / GCNT)
            nc.vector.tensor_mul(out=T8, in0=SG[:, 0:4], in1=SG[:, 0:4])
            nc.vector.tensor_tensor(out=SG[:, 4:8], in0=SG[:, 4:8], in1=T8,
                                    op=OP.subtract)
            # rstd = (var + eps) ** -0.5 via DVE fast-rsqrt + 1 Newton step
            nc.vector.tensor_scalar_add(out=SG[:, 4:8], in0=SG[:, 4:8],
                                        scalar1=EPS)
            Y8 = spool.tile([8, 4], F32, tag="y8", name="y8", bufs=4)
            vi = SG[:, 4:8].bitcast(mybir.dt.uint32)
            yi = Y8.bitcast(mybir.dt.uint32)
            nc.vector.tensor_scalar(out=yi, in0=vi, scalar1=1, scalar2=None,
                                    op0=OP.logical_shift_right)
            nc.vector.tensor_scalar(out=yi, in0=yi, scalar1=-1,
                                    scalar2=0x5F3759DF, op0=OP.mult, op1=OP.add)
            nc.vector.tensor_mul(out=T8, in0=Y8, in1=Y8)
            nc.vector.tensor_mul(out=T8, in0=T8, in1=SG[:, 4:8])
            nc.vector.tensor_scalar(out=T8, in0=T8, scalar1=-0.5,
                                    scalar2=1.5, op0=OP.mult, op1=OP.add)
            nc.vector.tensor_mul(out=Y8, in0=Y8, in1=T8)
            nc.vector.tensor_copy(out=SG[:, 4:8], in_=Y8)
            MBp = small_ps()
            nc.tensor.matmul(MBp[:, :8], ATM, SG, start=True, stop=True)
            MB = spool.tile([128, 8], F32, tag="mb", name="mb", bufs=4)
            nc.vector.tensor_copy(out=MB, in_=MBp[:, :8])
            SC = spool.tile([128, 4], F32, tag="sc", name="sc", bufs=4)
            TC = spool.tile([128, 4], F32, tag="tc", name="tc", bufs=4)
            nc.vector.tensor_mul(out=SC, in0=MB[:, 4:8], in1=CT[:, gcol:gcol + 4])
            nc.vector.tensor_mul(out=TC, in0=MB[:, 0:4], in1=SC)
            nc.vector.tensor_tensor(out=TC, in0=CT[:, bcol:bcol + 4], in1=TC,
                                    op=OP.subtract)
            return SC, TC

        def gn_y(gkey, src, ss, ring0=False):
            """GN over conv-output frame src (sums from stage2 accum);
            SiLU applied in place on the valid region. ring0: frame ring is
            known-zero, so the square pass can read the frame contiguously."""
            ST = new_st()
            nc.vector.tensor_reduce(out=ST[:, 0:4], in_=ss, axis=AX.X,
                                    op=OP.add)
            for chi in range(NCHI):
                if ring0:
                    nc.scalar.activation(
                        out=GAR, in_=src[:, _fb(chi):_fb(chi) + FR],
                        func=AF.Square,
                        accum_out=ST[:, 4 + chi: 5 + chi])
                else:
                    nc.scalar.activation(out=garv, in_=_valid(src, chi),
                                         func=AF.Square,
                                         accum_out=ST[:, 4 + chi: 5 + chi])
            SC, TC = gn_finalize(gkey, ST)
            for chi in range(NCHI):
                nc.scalar.activation(out=_valid(src, chi),
                                     in_=_valid(src, chi), func=AF.Silu,
                                     bias=TC[:, chi:chi + 1],
                                     scale=SC[:, chi:chi + 1])

        def gn_att(img, STa):
            """GN over XF (sums prefilled in STa) -> linear hc for qkv."""
            for chi in range(NCHI):
                nc.scalar.activation(out=garv, in_=_valid(XF[img], chi),
                                     func=AF.Square,
                                     accum_out=STa[:, 4 + chi: 5 + chi])
            SC, TC = gn_finalize("att", STa)
            hc = wpool.tile([128, NCHI, 1024], F16, tag=f"w{img}",
                            name=f"hc{img}")
            for chi in range(NCHI):
                nc.vector.tensor_scalar(
                    out=hc[:, chi].rearrange("p (r w) -> p r w", w=32),
                    in0=_valid(XF[img], chi),
                    scalar1=SC[:, chi:chi + 1], scalar2=TC[:, chi:chi + 1],
                    op0=OP.mult, op1=OP.add)
            return hc

        def new_st():
            return spool.tile([128, 8], F32, tag="st", name="st", bufs=4)

        def new_ss(nm):
            return spool.tile([128, 4, 4], F32, tag="ss", name=nm, bufs=2)

        def new_y(nm):
            return spool.tile([128, 4, 4, 256], F16, tag="ylin", name=nm,
                              bufs=2)

        def xf_add(img, sf, STa=None):
            """XF[img] += sf (valid region); optional fused GN sums."""
            for chi in range(NCHI):
                ov = _valid(XF[img], chi)
                if STa is not None:
                    nc.vector.scalar_tensor_tensor(
                        out=ov, in0=ov, scalar=1.0, in1=_valid(sf, chi),
                        op0=OP.mult, op1=OP.add,
                        accum_out=STa[:, chi:chi + 1])
                else:
                    nc.vector.tensor_add(out=ov, in0=ov, in1=_valid(sf, chi))

        # ---------------- attention ----------------
        def att_qkv(img, hc):
            Q = scr.tile([128, NCHI, 1024], F16, tag=f"fa{img}", name=f"q{img}")
            K = scr.tile([128, NCHI, 1024], F16, tag="cpl", name=f"k{img}")
            V8 = apool.tile([128, 8, 512], F16, tag="v", name=f"v{img}")
            for which, dst, bcol in ((0, Q, QB_COL), (1, K, KB_COL)):
                for co in range(NCHI):
                    for ns in range(2):
                        ps = ps_slot()
                        for chi in range(NCHI):
                            nc.tensor.matmul(
                                ps, WA[:, which, chi, bass.ts(co, 128)],
                                hc[:, chi, bass.ts(ns, 512)],
                                start=chi == 0, stop=chi == NCHI - 1)
                        nc.vector.tensor_scalar_add(
                            out=dst[:, co, bass.ts(ns, 512)], in0=ps,
                            scalar1=CT[:, bcol + co: bcol + co + 1])
            for nb in range(8):
                ps = ps_slot()
                for chi in range(NCHI):
                    nc.tensor.matmul(ps, hc[:, chi, bass.ts(nb, 128)],
                                     WA[:, 2, chi, :],
                                     start=chi == 0, stop=chi == NCHI - 1)
                if nb % 2 == 0:
                    nc.vector.tensor_copy(out=V8[:, nb, :], in_=ps)
                else:
                    nc.scalar.copy(out=V8[:, nb, :], in_=ps)
            return Q, K, V8

        def att_core(img, Q, K, V8, mid=None):
            # transposed scores: AT[kv-part, q] = exp(K^T Q); row sums via
            # ones-matmul (broadcast over partitions); scale by reciprocal.
            AT = vpool.tile([128, 8, 1024], F16, tag=f"vp{img}",
                            name=f"at{img}")
            RB = spool.tile([128, 1024], F32, tag="rb", name=f"rb{img}",
                            bufs=1)
            sums = [small_ps(), small_ps()]
            for kb in range(8):
                for qh in range(2):
                    ps = ps_slot()
                    for chi in range(NCHI):
                        nc.tensor.matmul(ps, K[:, chi, bass.ts(kb, 128)],
                                         Q[:, chi, bass.ts(qh, 512)],
                                         start=chi == 0, stop=chi == NCHI - 1)
                    # scores are tiny (~N(0,0.04)): skip max-subtraction.
                    nc.scalar.activation(out=AT[:, kb, bass.ts(qh, 512)],
                                         in_=ps, func=AF.Exp)
                    nc.tensor.matmul(sums[qh], ONES,
                                     AT[:, kb, bass.ts(qh, 512)],
                                     start=kb == 0, stop=kb == 7)
            for qh in range(2):
                nc.vector.reciprocal_approx_fast(
                    out=RB[:, bass.ts(qh, 512)], in_=sums[qh])
            for kb in range(8):
                for qh in range(2):
                    eng = nc.vector if (kb + qh) % 2 == 0 else nc.gpsimd
                    eng.tensor_mul(out=AT[:, kb, bass.ts(qh, 512)],
                                   in0=AT[:, kb, bass.ts(qh, 512)],
                                   in1=RB[:, bass.ts(qh, 512)])
            if mid is not None:
                mid()
            HA = apool.tile([128, NCHI, 1024], F16, tag="ha", name=f"ha{img}")
            for cb in range(NCHI):
                for ms in range(2):
                    ps = ps_slot()
                    for nb in range(8):
                        nc.tensor.matmul(ps, V8[:, nb, bass.ts(cb, 128)],
                                         AT[:, nb, bass.ts(ms, 512)],
                                         start=nb == 0, stop=nb == 7)
                    nc.vector.tensor_scalar_add(
                        out=HA[:, cb, bass.ts(ms, 512)], in0=ps,
                        scalar1=CT[:, VB_COL + cb: VB_COL + cb + 1])
            for co in range(NCHI):
                for ms in range(2):
                    ps = ps_slot()
                    for chi in range(NCHI):
                        nc.tensor.matmul(ps, WA[:, 3, chi, bass.ts(co, 128)],
                                         HA[:, chi, bass.ts(ms, 512)],
                                         start=chi == 0, stop=chi == NCHI - 1)
                    r0 = 16 * ms + 1
                    ov = XF[img][:, _fb(co) + FW * r0: _fb(co) + FW * r0
                                 + 16 * FW]
                    ov = ov.rearrange("p (r w) -> p r w", w=FW)[:, :, 1:33]
                    nc.vector.scalar_tensor_tensor(
                        out=ov, in0=ps.rearrange("p (r w) -> p r w", w=32),
                        scalar=CT[:, PB_COL + co: PB_COL + co + 1], in1=ov,
                        op0=OP.add, op1=OP.add)

        def frame(img, nm):
            return scr.tile([128, PFREE], F16, tag=f"fa{img}", name=nm)

        # ================= emission schedule =================
        VP = [vpool.tile([128, 4, 4096], F16, tag=f"vp{b}",
                         name=f"vp_r1c1_{b}") for b in range(B_LOC)]
        make_V(0, XF[0], VP[0], "r1c1")
        make_V(1, XF[1], VP[1], "r1c1")

        def resnet(blk, vnext):
            """vnext(img, sf) emitted per image after conv2+gn2."""
            c1, c2 = f"{blk}c1", f"{blk}c2"
            for img in range(B_LOC):
                h1 = frame(img, f"h1_{blk}{img}")
                ring_zero(h1)
                ssx = new_ss(f"ss_{c1}{img}")
                conv_img(c1, img, VP[img], h1, ssx)
                gn_y(f"{blk}g1", h1, ssx, ring0=True)
                nv = vpool.tile([128, 4, 4096], F16, tag=f"vp{img}",
                                name=f"vp_{c2}_{img}")
                make_V(img, h1, nv, c2)
                VP[img] = nv
            sf = [None, None]
            ss2 = [None, None]
            for img in range(B_LOC):
                sf[img] = frame(img, f"sf_{blk}{img}")
                ss2[img] = new_ss(f"ss_{c2}{img}")
                conv_img(c2, img, VP[img], sf[img], ss2[img])
                if img == 0:
                    gn_y(f"{blk}g2", sf[0], ss2[0])
            # vnext(0) is emitted after conv2(img1)'s weight DMAs so tiles it
            # places in the w0/w1 tag slots (gn_att's hc) cannot deadlock the
            # weight-slot rotation.
            vnext(0, sf[0])
            gn_y(f"{blk}g2", sf[1], ss2[1])
            vnext(1, sf[1])

        # ---- r1 + attention interleave ----
        hcs = [None, None]

        def r1_next(img, sf):
            STa = new_st()
            xf_add(img, sf, STa)
            hcs[img] = gn_att(img, STa)

        resnet("r1", r1_next)
        qkv0 = att_qkv(0, hcs[0])
        att_core(0, *qkv0)
        qkv1 = att_qkv(1, hcs[1])
        nv0 = vpool.tile([128, 4, 4096], F16, tag="vp0", name="vp_r2c1_0")
        att_core(1, *qkv1, mid=lambda: make_V(0, XF[0], nv0, "r2c1"))
        VP[0] = nv0
        nv1 = vpool.tile([128, 4, 4096], F16, tag="vp1", name="vp_r2c1_1")
        make_V(1, XF[1], nv1, "r2c1")
        VP[1] = nv1

        # ---- r2 ----
        def r2_next(img, sf):
            xf_add(img, sf, None)
            eng = nc.sync if img == 0 else nc.gpsimd
            for chi in range(NCHI):
                eng.dma_start(out=out_d[:, img, chi, :],
                              in_=_valid(XF[img], chi))

        resnet("r2", r2_next)

    nc.compile()
    return nc


def _prep_inputs(inputs):
    f32 = np.float32
    f16 = np.float16
    x = np.asarray(inputs["x"], f32)
    xp = np.zeros((N_CORES, B_LOC, NCHI, 128, FW, FW), f32)
    xp[:, :, :, :, 1:33, 1:33] = x.reshape(N_CORES, B_LOC, NCHI, 128, 32, 32)
    x_pad = np.ascontiguousarray(
        xp.transpose(0, 3, 1, 2, 4, 5).reshape(N_CORES, 128, B_LOC, PFREE)
    ).astype(f16)

    Gm = np.array([[1, 0, 0], [.5, .5, .5], [.5, -.5, .5], [0, 0, 1]], f32)

    def winow(w):
        w = np.asarray(w, f32)  # [co, ci, 3, 3]
        U = np.einsum('ak,oikl,el->aeoi', Gm, w, Gm, optimize=True)
        # [a, e, o, i] -> [p, cob, h, 2a+bb, chi, cof], pos = 4a + 2h + bb
        U = U.reshape(4, 2, 2, 4, 128, NCHI, 128)  # [a, h, bb, cob, cof, chi, p]
        arr = U.transpose(6, 3, 1, 0, 2, 5, 4)  # [p, cob, h, a, bb, chi, cof]
        return np.ascontiguousarray(
            arr.reshape(128, 4, 2, 8, NCHI, 128)).astype(f16)

    def onew(w):
        return np.ascontiguousarray(
            np.asarray(w, f32).T.reshape(NCHI, 128, C).transpose(1, 0, 2))

    def col(v):
        return np.asarray(v, f32).reshape(NCHI, 128).T

    scale = C ** -0.5
    wq = onew(np.asarray(inputs["a_qw"], f32) * scale)
    wk, wv, wp = onew(inputs["a_kw"]), onew(inputs["a_vw"]), onew(inputs["a_pw"])
    wqkvp = np.ascontiguousarray(np.stack([wq, wk, wv, wp], axis=1)).astype(f16)

    ct = np.zeros((128, 80), np.float32)
    ct[:, 0:4] = col(inputs["r1_c1b"])
    ct[:, 4:8] = col(inputs["r1_c2b"])
    ct[:, 8:12] = col(inputs["r2_c1b"])
    ct[:, 12:16] = col(inputs["r2_c2b"])
    for (g, bta), (gc, bc) in zip(
            [("r1_g1", "r1_b1"), ("r1_g2", "r1_b2"), ("a_g", "a_b"),
             ("r2_g1", "r2_b1"), ("r2_g2", "r2_b2")],
            [GN_COLS[k] for k in ("r1g1", "r1g2", "att", "r2g1", "r2g2")]):
        ct[:, gc:gc + 4] = col(inputs[g])
        ct[:, bc:bc + 4] = col(inputs[bta])
    p_idx = np.arange(128)
    ct[:, A_COL:A_COL + 8] = (p_idx[:, None] // 16 == np.arange(8)[None, :])
    ct[:, QB_COL:QB_COL + 4] = col(np.asarray(inputs["a_qb"], f32) * scale)
    ct[:, KB_COL:KB_COL + 4] = col(inputs["a_kb"])
    ct[:, VB_COL:VB_COL + 4] = col(inputs["a_vb"])
    ct[:, PB_COL:PB_COL + 4] = col(inputs["a_pb"])
    atm = np.ascontiguousarray(
        (np.arange(8)[:, None] == p_idx[None, :] // 16).astype(np.float32))

    shared = {
        "w_r1c1": winow(inputs["r1_c1w"]), "w_r1c2": winow(inputs["r1_c2w"]),
        "w_r2c1": winow(inputs["r2_c1w"]), "w_r2c2": winow(inputs["r2_c2w"]),
        "wqkvp": wqkvp, "consts": ct, "atm": atm,
    }
    in_maps = [dict(shared, x_pad=np.ascontiguousarray(x_pad[i]))
               for i in range(N_CORES)]
    return in_maps


_NC_CACHE = {}


def _get_nc(num_devices=N_CORES):
    key = num_devices
    if key not in _NC_CACHE:
        _NC_CACHE[key] = _build(num_devices)
    return _NC_CACHE[key]


def _gather(results):
    outs = [r["out"] for r in results]  # each [128, B_LOC, NCHI, 1024] f16
    y = np.stack(outs, axis=0)  # [8, 128, 2, 4, 1024]
    y = y.astype(np.float32).transpose(0, 2, 3, 1, 4).reshape(B, C, HH, WW)
    return np.ascontiguousarray(y)


def kernel(**inputs):
    nc = _get_nc()
    in_maps = _prep_inputs(inputs)
    res = run_bass_kernel_spmd(nc, in_maps, core_ids=list(range(N_CORES)))
    return _gather(res.results)


# revision 29
# speedup vs baseline: 1.0725x; 1.0725x over previous
"""Trainium2 Bass kernel for nn_MidBlock (ResNet -> Attention -> ResNet).

Data-parallel over batch: 16 images -> 8 cores x 2 images.
Convs use Winograd F(2x2,3x3): conv3x3 becomes 16 per-tile-position
[C_in x C_out] matmuls over 256 tiles/image, cutting tensor-engine
columns 2.4x vs direct conv. All matmul data in fp16 (fp32 accumulate).
Winograd input transforms run on DVE via strided views; the output
transform drains PSUM through ACT copies + DVE combines into a
contiguous linear Y buffer (GroupNorm sums fused via accum_out), and
the GroupNorm SiLU on ACT scatters Y back into padded-frame layout.
Softmax runs on transposed scores (kv on partitions): row sums come
from a ones-matmul broadcast, so no PE transposes are needed.
"""

import contextlib

import numpy as np

import concourse.bacc as bacc
import concourse.bass as bass
import concourse.tile as tile
from concourse import mybir
from concourse.bass_utils import run_bass_kernel_spmd

F32 = mybir.dt.float32
F16 = mybir.dt.float16
AF = mybir.ActivationFunctionType
OP = mybir.AluOpType
AX = mybir.AxisListType

N_CORES = 8
C = 512
B = 16
HH = 32
WW = 32
B_LOC = B // N_CORES  # 2 images per core
NCHI = 4  # channel blocks of 128
FW = 34  # padded frame width
FR = FW * FW  # 1156
PFREE = NCHI * FR  # 4624
EPS = 1e-6
GCNT = 16 * HH * WW  # elements per group

# consts tile column map (CT [128, 80] fp32)
CB = {"r1c1": 0, "r1c2": 4, "r2c1": 8, "r2c2": 12}
GN_COLS = {"r1g1": (16, 20), "r1g2": (24, 28), "att": (32, 36),
           "r2g1": (40, 44), "r2g2": (48, 52)}
A_COL = 56
QB_COL, KB_COL, VB_COL, PB_COL = 64, 68, 72, 76


def _fb(chi):
    return chi * FR


def _valid(t, chi):
    """[128, 32, 32] view of valid pixels of frame chi in per-image tile t."""
    s = t[:, _fb(chi) + FW: _fb(chi) + FW + 32 * FW]
    return s.rearrange("p (r w) -> p r w", w=FW)[:, :, 1:33]


def _build(num_devices):
    nc = bacc.Bacc("TRN2", target_bir_lowering=False, debug=False,
                   num_devices=num_devices)
    x_pad = nc.dram_tensor("x_pad", [128, B_LOC, PFREE], F16,
                           kind="ExternalInput").ap()
    wc = {k: nc.dram_tensor(f"w_{k}", [128, 4, 2, 8, NCHI, 128], F16,
                            kind="ExternalInput").ap()
          for k in ("r1c1", "r1c2", "r2c1", "r2c2")}
    wqkvp = nc.dram_tensor("wqkvp", [128, 4, NCHI, C], F16,
                           kind="ExternalInput").ap()
    ct_d = nc.dram_tensor("consts", [128, 80], F32, kind="ExternalInput").ap()
    atm_d = nc.dram_tensor("atm", [8, 128], F32, kind="ExternalInput").ap()
    out_d = nc.dram_tensor("out", [128, B_LOC, NCHI, 1024], F16,
                           kind="ExternalOutput").ap()

    with tile.TileContext(nc) as tc, contextlib.ExitStack() as ctx:
        pers = ctx.enter_context(tc.tile_pool(name="pers", bufs=1))
        scr = ctx.enter_context(tc.tile_pool(name="scr", bufs=1))
        wpool = ctx.enter_context(tc.tile_pool(name="wpool", bufs=1))
        cpool = ctx.enter_context(tc.tile_pool(name="cpool", bufs=1))
        spool = ctx.enter_context(tc.tile_pool(name="spool", bufs=1))
        apool = ctx.enter_context(tc.tile_pool(name="apool", bufs=1))
        vpool = ctx.enter_context(tc.tile_pool(name="vpool", bufs=1))
        psum = ctx.enter_context(tc.tile_pool(name="psum", bufs=1, space="PSUM"))

        psctr = [0]

        def ps_slot():
            t = psum.tile([128, 512], F32, tag=f"m{psctr[0] % 6}",
                          name=f"ps{psctr[0]}")
            psctr[0] += 1
            return t

        def small_ps():
            return psum.tile([128, 512], F32, tag="tp", name="tp", bufs=2)

        # ---- persistent residual frames + input DMAs ----
        XF = [pers.tile([128, PFREE], F16, tag=f"xf{b}", name=f"xf{b}")
              for b in range(B_LOC)]
        for b, eng in ((0, nc.sync), (1, nc.gpsimd)):
            for chi in range(NCHI):
                eng.dma_start(out=XF[b][:, _fb(chi):_fb(chi) + FR],
                              in_=x_pad[:, b, _fb(chi):_fb(chi) + FR])

        CT = cpool.tile([128, 80], F32, tag="ct", name="ct")
        nc.sync.dma_start(out=CT, in_=ct_d)
        ATM = cpool.tile([8, 128], F32, tag="atm", name="atm")
        nc.sync.dma_start(out=ATM, in_=atm_d)
        WA = cpool.tile([128, 4, NCHI, C], F16, tag="wqkvp", name="wqkvp")
        nc.gpsimd.dma_start(out=WA, in_=wqkvp)
        ONES = cpool.tile([128, 128], F16, tag="ones", name="ones")
        nc.vector.memset(ONES, 1.0)
        GAR = scr.tile([128, 1024], F16, tag="gar", name="gar")
        garv = GAR.rearrange("p (r w) -> p r w", w=32)
        gary = GAR.rearrange("p (a j) -> p a j", a=4)

        wctr = [0]
        wdmae = [nc.sync, nc.gpsimd]

        # ---------------- Winograd input transform ----------------
        # c-planes stored [t][(c i)][bb][j]: row-parity-major so every view
        # below is a legal 2-free-dim AP. V-planes stored [a][(chi i bb j)]
        # (bb interleaved); the matmul reads strided [i, j] windows.
        def make_V(img, src, vp, pfx):
            cp = scr.tile([128, 2, 4352], F16, tag="cpl",
                          name=f"cp_{pfx}{img}")
            sv = src[:, 0:NCHI * FR].rearrange(
                "p (ci t j s) -> p ci t j s", ci=4 * 17, t=2, j=17, s=2)
            for t in range(2):
                A0 = sv[:, :, t, 0:16, 0]   # col 2j
                A1 = sv[:, :, t, 1:17, 0]   # col 2j+2
                B0 = sv[:, :, t, 0:16, 1]   # col 2j+1
                B1 = sv[:, :, t, 1:17, 1]   # col 2j+3

                def cv(n, t=t):
                    return cp[:, t].rearrange(
                        "p (ci bb j) -> p ci bb j", ci=68, bb=4,
                        j=16)[:, :, n, :]

                nc.vector.tensor_sub(out=cv(0), in0=A0, in1=A1)
                nc.vector.tensor_add(out=cv(1), in0=B0, in1=A1)
                nc.vector.tensor_sub(out=cv(2), in0=A1, in1=B0)
                nc.vector.tensor_sub(out=cv(3), in0=B0, in1=B1)
            # H-direction: rows 2ti(+1,+2,+3) = (t, i windows); all four
            # bb planes and all chi processed in one op per output row a.
            ce = [cp[:, t].rearrange("p (c ibj) -> p c ibj", c=4, ibj=1088)
                  for t in range(2)]
            E0 = ce[0][:, :, 0:1024]
            E1 = ce[0][:, :, 64:1088]
            O0 = ce[1][:, :, 0:1024]
            O1 = ce[1][:, :, 64:1088]

            def vv(a):
                return vp[:, a].rearrange("p (c ibj) -> p c ibj", c=4,
                                          ibj=1024)

            nc.vector.tensor_sub(out=vv(0), in0=E0, in1=E1)
            nc.vector.tensor_add(out=vv(1), in0=O0, in1=E1)
            nc.vector.tensor_sub(out=vv(2), in0=E1, in1=O0)
            nc.vector.tensor_sub(out=vv(3), in0=O0, in1=O1)

        def vrhs(vp, a, bb, chi):
            """[128,16,16] strided matmul rhs for tile position (a,bb)."""
            return vp[:, a].rearrange("p (c i bb j) -> p c i bb j",
                                      c=4, i=16, bb=4, j=16)[:, chi, :, bb, :]

        def frame_view(f, co, p, q):
            """[128,16,16] strided view of output positions (p,q) of tiles."""
            f5 = f[:, _fb(co):_fb(co) + FR].rearrange(
                "pp (i t j s) -> pp i t j s", i=17, t=2, j=17, s=2)
            ri = slice(0, 16) if p == 0 else slice(1, 17)
            rj = slice(0, 16) if q == 0 else slice(1, 17)
            return f5[:, ri, 1 - p, rj, 1 - q]

        def ring_zero(f):
            """Zero the 34x34 padding ring of each chi frame (Pool engine)."""
            for chi in range(NCHI):
                base = _fb(chi)
                nc.gpsimd.memset(f[:, base:base + FW], 0.0)
                nc.gpsimd.memset(f[:, base + 33 * FW:base + 34 * FW], 0.0)
                colv = f[:, base + FW:base + 33 * FW].rearrange(
                    "p (r w) -> p r w", w=FW)
                nc.gpsimd.memset(colv[:, :, 0:1], 0.0)
                nc.gpsimd.memset(colv[:, :, 33:34], 0.0)

        # ---------------- Winograd conv (one image) ----------------
        # Output goes to a contiguous linear buffer Y [128, co, 2q+p, 256];
        # the GN SiLU pass (ACT) scatters it into frame layout afterwards.
        def conv_img(key, img, vp, outf, ss):
            bcol = CB[key]
            for co in range(4):
                tp_t = spool.tile([128, 2, 4, 256], F16, tag="tpn",
                                  name=f"t_{key}{img}{co}", bufs=2)
                for h in range(2):  # b-pair: two accum groups share a bank
                    w = wpool.tile([128, 8, NCHI, 128], F16,
                                   tag=f"w{wctr[0] % 2}",
                                   name=f"w_{key}{img}{co}{h}")
                    wdmae[wctr[0] % 2].dma_start(out=w, in_=wc[key][:, co, h])
                    wctr[0] += 1
                    slots = [ps_slot() for _ in range(4)]
                    for a in range(4):
                        for bb in range(2):
                            b = 2 * h + bb
                            for chi in range(NCHI):
                                nc.tensor.matmul(
                                    slots[a][:, bb * 256:(bb + 1) * 256],
                                    w[:, 2 * a + bb, chi, :],
                                    vrhs(vp, a, b, chi),
                                    start=chi == 0, stop=chi == 3)
                    sl = [s.rearrange("pp (a j) -> pp a j", a=2)
                          for s in slots]
                    # DVE reads at most one PSUM input per op: stage m1
                    # through an ACT copy to SBUF.
                    uu = spool.tile([128, 2, 256], F16, tag="u1",
                                    name=f"u_{key}{img}{co}{h}", bufs=2)
                    nc.scalar.copy(out=uu, in_=sl[1])
                    t0 = tp_t[:, 0, 2 * h:2 * h + 2, :]
                    t1 = tp_t[:, 1, 2 * h:2 * h + 2, :]
                    nc.vector.tensor_add(out=t0, in0=uu, in1=sl[0])
                    nc.vector.tensor_add(out=t0, in0=t0, in1=sl[2])
                    nc.vector.tensor_sub(out=t1, in0=uu, in1=sl[2])
                    nc.vector.tensor_sub(out=t1, in0=t1, in1=sl[3])
                bias = CT[:, bcol + co: bcol + co + 1]
                for q in range(2):
                    yt = spool.tile([128, 2, 256], F16, tag="yt",
                                    name=f"yt_{key}{img}{co}{q}", bufs=2)
                    if q == 0:
                        nc.vector.tensor_add(out=yt, in0=tp_t[:, 0:2, 0, :],
                                             in1=tp_t[:, 0:2, 1, :])
                    else:
                        nc.vector.tensor_sub(out=yt, in0=tp_t[:, 0:2, 1, :],
                                             in1=tp_t[:, 0:2, 2, :])
                    b3 = 2 if q == 0 else 3
                    op1 = OP.add if q == 0 else OP.subtract
                    for p in range(2):
                        k = 2 * q + p
                        ytv = yt[:, p].rearrange("pp (i j) -> pp i j", j=16)
                        t3 = tp_t[:, p, b3, :].rearrange(
                            "pp (i j) -> pp i j", j=16)
                        nc.vector.scalar_tensor_tensor(
                            out=frame_view(outf, co, p, q), in0=ytv,
                            scalar=bias, in1=t3, op0=OP.add, op1=op1,
                            accum_out=ss[:, co, k:k + 1])

        # ---------------- group norm ----------------
        def gn_finalize(gkey, ST):
            """Common GN tail: ST = [sums|sumsq] -> per-channel (s, t)."""
            gcol, bcol = GN_COLS[gkey]
            G = small_ps()
            nc.tensor.matmul(G[:8, :8], CT[:, A_COL:A_COL + 8], ST, start=True,
                             stop=True)
            SG = spool.tile([8, 8], F32, tag="sg", name="sg", bufs=4)
            T8 = spool.tile([8, 4], F32, tag="t8", name="t8", bufs=4)
            nc.vector.tensor_scalar_mul(out=SG, in0=G[:8, :8],
                                        scalar1=1.0 / GCNT)
            nc.vector.tensor_mul(out=T8, in0=SG[:, 0:4], in1=SG[:, 0:4])
            nc.vector.tensor_tensor(out=SG[:, 4:8], in0=SG[:, 4:8], in1=T8,
                                    op=OP.subtract)
            # rstd = (var + eps) ** -0.5 via DVE fast-rsqrt + 1 Newton step
            nc.vector.tensor_scalar_add(out=SG[:, 4:8], in0=SG[:, 4:8],
                                        scalar1=EPS)
            Y8 = spool.tile([8, 4], F32, tag="y8", name="y8", bufs=4)
            vi = SG[:, 4:8].bitcast(mybir.dt.uint32)
            yi = Y8.bitcast(mybir.dt.uint32)
            nc.vector.tensor_scalar(out=yi, in0=vi, scalar1=1, scalar2=None,
                                    op0=OP.logical_shift_right)
            nc.vector.tensor_scalar(out=yi, in0=yi, scalar1=-1,
                                    scalar2=0x5F3759DF, op0=OP.mult, op1=OP.add)
            nc.vector.tensor_mul(out=T8, in0=Y8, in1=Y8)
            nc.vector.tensor_mul(out=T8, in0=T8, in1=SG[:, 4:8])
            nc.vector.tensor_scalar(out=T8, in0=T8, scalar1=-0.5,
                                    scalar2=1.5, op0=OP.mult, op1=OP.add)
            nc.vector.tensor_mul(out=Y8, in0=Y8, in1=T8)
            nc.vector.tensor_copy(out=SG[:, 4:8], in_=Y8)
            MBp = small_ps()
            nc.tensor.matmul(MBp[:, :8], ATM, SG, start=True, stop=True)
            MB = spool.tile([128, 8], F32, tag="mb", name="mb", bufs=4)
            nc.vector.tensor_copy(out=MB, in_=MBp[:, :8])
            SC = spool.tile([128, 4], F32, tag="sc", name="sc", bufs=4)
            TC = spool.tile([128, 4], F32, tag="tc", name="tc", bufs=4)
            nc.vector.tensor_mul(out=SC, in0=MB[:, 4:8], in1=CT[:, gcol:gcol + 4])
            nc.vector.tensor_mul(out=TC, in0=MB[:, 0:4], in1=SC)
            nc.vector.tensor_tensor(out=TC, in0=CT[:, bcol:bcol + 4], in1=TC,
                                    op=OP.subtract)
            return SC, TC

        def gn_y(gkey, src, ss):
            """GN over conv-output frame src (sums from stage2 accum);
            SiLU applied in place on the valid region."""
            ST = new_st()
            nc.vector.tensor_reduce(out=ST[:, 0:4], in_=ss, axis=AX.X,
                                    op=OP.add)
            for chi in range(NCHI):
                nc.scalar.activation(out=garv, in_=_valid(src, chi),
                                     func=AF.Square,
                                     accum_out=ST[:, 4 + chi: 5 + chi])
            SC, TC = gn_finalize(gkey, ST)
            for chi in range(NCHI):
                nc.scalar.activation(out=_valid(src, chi),
                                     in_=_valid(src, chi), func=AF.Silu,
                                     bias=TC[:, chi:chi + 1],
                                     scale=SC[:, chi:chi + 1])

        def gn_att(img, STa):
            """GN over XF (sums prefilled in STa) -> linear hc for qkv."""
            for chi in range(NCHI):
                nc.scalar.activation(out=garv, in_=_valid(XF[img], chi),
                                     func=AF.Square,
                                     accum_out=STa[:, 4 + chi: 5 + chi])
            SC, TC = gn_finalize("att", STa)
            hc = wpool.tile([128, NCHI, 1024], F16, tag=f"w{img}",
                            name=f"hc{img}")
            for chi in range(NCHI):
                nc.vector.tensor_scalar(
                    out=hc[:, chi].rearrange("p (r w) -> p r w", w=32),
                    in0=_valid(XF[img], chi),
                    scalar1=SC[:, chi:chi + 1], scalar2=TC[:, chi:chi + 1],
                    op0=OP.mult, op1=OP.add)
            return hc

        def new_st():
            return spool.tile([128, 8], F32, tag="st", name="st", bufs=4)

        def new_ss(nm):
            return spool.tile([128, 4, 4], F32, tag="ss", name=nm, bufs=2)

        def new_y(nm):
            return spool.tile([128, 4, 4, 256], F16, tag="ylin", name=nm,
                              bufs=2)

        def xf_add(img, sf, STa=None):
            """XF[img] += sf (valid region); optional fused GN sums."""
            for chi in range(NCHI):
                ov = _valid(XF[img], chi)
                if STa is not None:
                    nc.vector.scalar_tensor_tensor(
                        out=ov, in0=ov, scalar=1.0, in1=_valid(sf, chi),
                        op0=OP.mult, op1=OP.add,
                        accum_out=STa[:, chi:chi + 1])
                else:
                    nc.vector.tensor_add(out=ov, in0=ov, in1=_valid(sf, chi))

        # ---------------- attention ----------------
        def att_qkv(img, hc):
            Q = scr.tile([128, NCHI, 1024], F16, tag=f"fa{img}", name=f"q{img}")
            K = scr.tile([128, NCHI, 1024], F16, tag="cpl", name=f"k{img}")
            V8 = apool.tile([128, 8, 512], F16, tag="v", name=f"v{img}")
            for which, dst, bcol in ((0, Q, QB_COL), (1, K, KB_COL)):
                for co in range(NCHI):
                    for ns in range(2):
                        ps = ps_slot()
                        for chi in range(NCHI):
                            nc.tensor.matmul(
                                ps, WA[:, which, chi, bass.ts(co, 128)],
                                hc[:, chi, bass.ts(ns, 512)],
                                start=chi == 0, stop=chi == NCHI - 1)
                        nc.vector.tensor_scalar_add(
                            out=dst[:, co, bass.ts(ns, 512)], in0=ps,
                            scalar1=CT[:, bcol + co: bcol + co + 1])
            for nb in range(8):
                ps = ps_slot()
                for chi in range(NCHI):
                    nc.tensor.matmul(ps, hc[:, chi, bass.ts(nb, 128)],
                                     WA[:, 2, chi, :],
                                     start=chi == 0, stop=chi == NCHI - 1)
                if nb % 2 == 0:
                    nc.vector.tensor_copy(out=V8[:, nb, :], in_=ps)
                else:
                    nc.scalar.copy(out=V8[:, nb, :], in_=ps)
            return Q, K, V8

        def att_core(img, Q, K, V8, mid=None):
            # transposed scores: AT[kv-part, q] = exp(K^T Q); row sums via
            # ones-matmul (broadcast over partitions); scale by reciprocal.
            AT = vpool.tile([128, 8, 1024], F16, tag=f"vp{img}",
                            name=f"at{img}")
            RB = spool.tile([128, 1024], F32, tag="rb", name=f"rb{img}",
                            bufs=1)
            sums = [small_ps(), small_ps()]
            for kb in range(8):
                for qh in range(2):
                    ps = ps_slot()
                    for chi in range(NCHI):
                        nc.tensor.matmul(ps, K[:, chi, bass.ts(kb, 128)],
                                         Q[:, chi, bass.ts(qh, 512)],
                                         start=chi == 0, stop=chi == NCHI - 1)
                    # scores are tiny (~N(0,0.04)): skip max-subtraction.
                    nc.scalar.activation(out=AT[:, kb, bass.ts(qh, 512)],
                                         in_=ps, func=AF.Exp)
                    nc.tensor.matmul(sums[qh], ONES,
                                     AT[:, kb, bass.ts(qh, 512)],
                                     start=kb == 0, stop=kb == 7)
            for qh in range(2):
                nc.vector.reciprocal_approx_fast(
                    out=RB[:, bass.ts(qh, 512)], in_=sums[qh])
            for kb in range(8):
                for qh in range(2):
                    eng = nc.vector if (kb + qh) % 2 == 0 else nc.gpsimd
                    eng.tensor_mul(out=AT[:, kb, bass.ts(qh, 512)],
                                   in0=AT[:, kb, bass.ts(qh, 512)],
                                   in1=RB[:, bass.ts(qh, 512)])
            if mid is not None:
                mid()
            HA = apool.tile([128, NCHI, 1024], F16, tag="ha", name=f"ha{img}")
            for cb in range(NCHI):
                for ms in range(2):
                    ps = ps_slot()
                    for nb in range(8):
                        nc.tensor.matmul(ps, V8[:, nb, bass.ts(cb, 128)],
                                         AT[:, nb, bass.ts(ms, 512)],
                                         start=nb == 0, stop=nb == 7)
                    nc.vector.tensor_scalar_add(
                        out=HA[:, cb, bass.ts(ms, 512)], in0=ps,
                        scalar1=CT[:, VB_COL + cb: VB_COL + cb + 1])
            for co in range(NCHI):
                for ms in range(2):
                    ps = ps_slot()
                    for chi in range(NCHI):
                        nc.tensor.matmul(ps, WA[:, 3, chi, bass.ts(co, 128)],
                                         HA[:, chi, bass.ts(ms, 512)],
                                         start=chi == 0, stop=chi == NCHI - 1)
                    r0 = 16 * ms + 1
                    ov = XF[img][:, _fb(co) + FW * r0: _fb(co) + FW * r0
                                 + 16 * FW]
                    ov = ov.rearrange("p (r w) -> p r w", w=FW)[:, :, 1:33]
                    nc.vector.scalar_tensor_tensor(
                        out=ov, in0=ps.rearrange("p (r w) -> p r w", w=32),
                        scalar=CT[:, PB_COL + co: PB_COL + co + 1], in1=ov,
                        op0=OP.add, op1=OP.add)

        def frame(img, nm):
            return scr.tile([128, PFREE], F16, tag=f"fa{img}", name=nm)

        # ================= emission schedule =================
        VP = [vpool.tile([128, 4, 4096], F16, tag=f"vp{b}",
                         name=f"vp_r1c1_{b}") for b in range(B_LOC)]
        make_V(0, XF[0], VP[0], "r1c1")
        make_V(1, XF[1], VP[1], "r1c1")

        def resnet(blk, vnext):
            """vnext(img, sf) emitted per image after conv2+gn2."""
            c1, c2 = f"{blk}c1", f"{blk}c2"
            for img in range(B_LOC):
                h1 = frame(img, f"h1_{blk}{img}")
                ring_zero(h1)
                ssx = new_ss(f"ss_{c1}{img}")
                conv_img(c1, img, VP[img], h1, ssx)
                gn_y(f"{blk}g1", h1, ssx)
                nv = vpool.tile([128, 4, 4096], F16, tag=f"vp{img}",
                                name=f"vp_{c2}_{img}")
                make_V(img, h1, nv, c2)
                VP[img] = nv
            sf = [None, None]
            ss2 = [None, None]
            for img in range(B_LOC):
                sf[img] = frame(img, f"sf_{blk}{img}")
                ss2[img] = new_ss(f"ss_{c2}{img}")
                conv_img(c2, img, VP[img], sf[img], ss2[img])
                if img == 0:
                    gn_y(f"{blk}g2", sf[0], ss2[0])
            # vnext(0) is emitted after conv2(img1)'s weight DMAs so tiles it
            # places in the w0/w1 tag slots (gn_att's hc) cannot deadlock the
            # weight-slot rotation.
            vnext(0, sf[0])
            gn_y(f"{blk}g2", sf[1], ss2[1])
            vnext(1, sf[1])

        # ---- r1 + attention interleave ----
        hcs = [None, None]

        def r1_next(img, sf):
            STa = new_st()
            xf_add(img, sf, STa)
            hcs[img] = gn_att(img, STa)

        resnet("r1", r1_next)
        qkv0 = att_qkv(0, hcs[0])
        att_core(0, *qkv0)
        qkv1 = att_qkv(1, hcs[1])
        nv0 = vpool.tile([128, 4, 4096], F16, tag="vp0", name="vp_r2c1_0")
        att_core(1, *qkv1, mid=lambda: make_V(0, XF[0], nv0, "r2c1"))
        VP[0] = nv0
        nv1 = vpool.tile([128, 4, 4096], F16, tag="vp1", name="vp_r2c1_1")
        make_V(1, XF[1], nv1, "r2c1")
        VP[1] = nv1

        # ---- r2 ----
        def r2_next(img, sf):
            xf_add(img, sf, None)
            eng = nc.sync if img == 0 else nc.gpsimd
            for chi in range(NCHI):
                eng.dma_start(out=out_d[:, img, chi, :],
                              in_=_valid(XF[img], chi))

        resnet("r2", r2_next)

    nc.compile()
    return nc


def _prep_inputs(inputs):
    f32 = np.float32
    f16 = np.float16
    x = np.asarray(inputs["x"], f32)
    xp = np.zeros((N_CORES, B_LOC, NCHI, 128, FW, FW), f32)
    xp[:, :, :, :, 1:33, 1:33] = x.reshape(N_CORES, B_LOC, NCHI, 128, 32, 32)
    x_pad = np.ascontiguousarray(
        xp.transpose(0, 3, 1, 2, 4, 5).reshape(N_CORES, 128, B_LOC, PFREE)
    ).astype(f16)

    Gm = np.array([[1, 0, 0], [.5, .5, .5], [.5, -.5, .5], [0, 0, 1]], f32)

    def winow(w):
        w = np.asarray(w, f32)  # [co, ci, 3, 3]
        U = np.einsum('ak,oikl,el->aeoi', Gm, w, Gm, optimize=True)
        # [a, e, o, i] -> [p, cob, h, 2a+bb, chi, cof], pos = 4a + 2h + bb
        U = U.reshape(4, 2, 2, 4, 128, NCHI, 128)  # [a, h, bb, cob, cof, chi, p]
        arr = U.transpose(6, 3, 1, 0, 2, 5, 4)  # [p, cob, h, a, bb, chi, cof]
        return np.ascontiguousarray(
            arr.reshape(128, 4, 2, 8, NCHI, 128)).astype(f16)

    def onew(w):
        return np.ascontiguousarray(
            np.asarray(w, f32).T.reshape(NCHI, 128, C).transpose(1, 0, 2))

    def col(v):
        return np.asarray(v, f32).reshape(NCHI, 128).T

    scale = C ** -0.5
    wq = onew(np.asarray(inputs["a_qw"], f32) * scale)
    wk, wv, wp = onew(inputs["a_kw"]), onew(inputs["a_vw"]), onew(inputs["a_pw"])
    wqkvp = np.ascontiguousarray(np.stack([wq, wk, wv, wp], axis=1)).astype(f16)

    ct = np.zeros((128, 80), np.float32)
    ct[:, 0:4] = col(inputs["r1_c1b"])
    ct[:, 4:8] = col(inputs["r1_c2b"])
    ct[:, 8:12] = col(inputs["r2_c1b"])
    ct[:, 12:16] = col(inputs["r2_c2b"])
    for (g, bta), (gc, bc) in zip(
            [("r1_g1", "r1_b1"), ("r1_g2", "r1_b2"), ("a_g", "a_b"),
             ("r2_g1", "r2_b1"), ("r2_g2", "r2_b2")],
            [GN_COLS[k] for k in ("r1g1", "r1g2", "att", "r2g1", "r2g2")]):
        ct[:, gc:gc + 4] = col(inputs[g])
        ct[:, bc:bc + 4] = col(inputs[bta])
    p_idx = np.arange(128)
    ct[:, A_COL:A_COL + 8] = (p_idx[:, None] // 16 == np.arange(8)[None, :])
    ct[:, QB_COL:QB_COL + 4] = col(np.asarray(inputs["a_qb"], f32) * scale)
    ct[:, KB_COL:KB_COL + 4] = col(inputs["a_kb"])
    ct[:, VB_COL:VB_COL + 4] = col(inputs["a_vb"])
    ct[:, PB_COL:PB_COL + 4] = col(inputs["a_pb"])
    atm = np.ascontiguousarray(
        (np.arange(8)[:, None] == p_idx[None, :] // 16).astype(np.float32))

    shared = {
        "w_r1c1": winow(inputs["r1_c1w"]), "w_r1c2": winow(inputs["r1_c2w"]),
        "w_r2c1": winow(inputs["r2_c1w"]), "w_r2c2": winow(inputs["r2_c2w"]),
        "wqkvp": wqkvp, "consts": ct, "atm": atm,
    }
    in_maps = [dict(shared, x_pad=np.ascontiguousarray(x_pad[i]))
               for i in range(N_CORES)]
    return in_maps


_NC_CACHE = {}


def _get_nc(num_devices=N_CORES):
    key = num_devices
    if key not in _NC_CACHE:
        _NC_CACHE[key] = _build(num_devices)
    return _NC_CACHE[key]


def _gather(results):
    outs = [r["out"] for r in results]  # each [128, B_LOC, NCHI, 1024] f16
    y = np.stack(outs, axis=0)  # [8, 128, 2, 4, 1024]
    y = y.astype(np.float32).transpose(0, 2, 3, 1, 4).reshape(B, C, HH, WW)
    return np.ascontiguousarray(y)


def kernel(**inputs):
    nc = _get_nc()
    in_maps = _prep_inputs(inputs)
    res = run_bass_kernel_spmd(nc, in_maps, core_ids=list(range(N_CORES)))
    return _gather(res.results)
